# revision 1
# baseline (speedup 1.0000x reference)
"""CIGLoss (segment_reduce) Trainium2 kernel.

Strategy (data-parallel over batch, per the sharding hint):
  - Each of the 8 NeuronCores owns one image and that image's pixel list
    (segments are image-local: seg // 500 == image).
  - Host-side sharding packs each image's ~500 segments into a
    [128 partitions, NSLOT slots, L] padded grid (one whole segment per
    slot).  Pad entries point at a zero element appended to the image, so
    they contribute 0 to every sum.
  - The value lookup input[b,0,row,col] happens during host packing (this
    toolchain's walrus mis-lowers per-element indirect DMA: one descriptor
    per contiguous dest run, only the run-start offset honored — verified
    by hardware probes; see hw_gather_probe*.py).  All reductions run on
    device, per-slot:
        sums  = reduce_add(v)            counts = reduce_add(row < H)
        mean  = sums * recip(max(counts,1))
        dev   = reduce_add(|v - mean|)   contrib = dev * recip
    and a final partition reduce to one scalar per core.
  - Host sums the 8 per-core partials and divides by B.
"""

import numpy as np

_NUM_PATHS = 4000
_P = 128  # SBUF partitions


def _build_nc(nslot: int, L: int, ntot: int, W: int, H: int, chunk: int):
    import concourse.bacc as bacc
    import concourse.bass as bass
    import concourse.tile as tile
    from concourse import mybir

    f32 = mybir.dt.float32
    i32 = mybir.dt.int32
    Alu = mybir.AluOpType
    Ax = mybir.AxisListType
    FREE = nslot * L

    assert L % chunk == 0 or chunk % L == 0
    nch = FREE // chunk
    spc = max(1, chunk // L)   # whole slots per chunk (when chunk >= L)
    cps = max(1, L // chunk)   # chunks per slot (when chunk < L)

    u8 = mybir.dt.uint8
    nc = bacc.Bacc("TRN2", debug=False)
    v_d = nc.dram_tensor("vP", [_P, FREE], f32, kind="ExternalInput")
    ind_d = nc.dram_tensor("indP", [_P, FREE], u8, kind="ExternalInput")
    out_d = nc.dram_tensor("out", [_P, 1], f32, kind="ExternalOutput")

    _emit(nc, tile, bass, nslot, L, W, H, chunk, f32, u8, Alu, Ax,
          v_d, ind_d, out_d, FREE, nch, spc, cps)
    # Bacc defers register allocation + wait-splitting to finalize(); the
    # pjrt run path serializes the module as-is, so finalize here.
    nc.finalize()
    return nc


def _emit(nc, tile, bass, nslot, L, W, H, chunk, f32, u8, Alu, Ax,
          v_d, ind_d, out_d, FREE, nch, spc, cps):
    with tile.TileContext(nc) as tc:
        with (
            tc.tile_pool(name="big", bufs=1) as big,
            tc.tile_pool(name="small", bufs=1) as small,
        ):
            # u8 indicator of real (non-pad) pixels, upcast to f32
            ind8_t = big.tile([_P, FREE], u8)
            nc.sync.dma_start(out=ind8_t[:], in_=ind_d[:, :])
            ind_t = big.tile([_P, FREE], f32)
            nc.vector.tensor_copy(out=ind_t[:], in_=ind8_t[:])

            # gathered pixel values in slot layout; chunked load with
            # per-chunk partial sums so load and reduce overlap.
            v_t = big.tile([_P, FREE], f32)
            psum_t = small.tile([_P, nch * spc], f32)
            for k in range(nch):
                a, b = k * chunk, (k + 1) * chunk
                nc.sync.dma_start(out=v_t[:, a:b], in_=v_d[:, a:b])
                nc.vector.tensor_reduce(
                    out=psum_t[:, k * spc:(k + 1) * spc],
                    in_=v_t[:, a:b].rearrange("p (s l) -> p s l", s=spc),
                    axis=Ax.X, op=Alu.add,
                )

            v3 = v_t[:].rearrange("p (s l) -> p s l", s=nslot)
            ind3 = ind_t[:].rearrange("p (s l) -> p s l", s=nslot)

            # combine per-chunk partials into per-slot sums
            sums = small.tile([_P, nslot], f32)
            if cps == 1:
                nc.vector.tensor_copy(out=sums[:], in_=psum_t[:])
            elif cps == 2:
                nc.vector.tensor_tensor(
                    out=sums[:], in0=psum_t[:, 0::2], in1=psum_t[:, 1::2],
                    op=Alu.add,
                )
            else:
                nc.vector.tensor_reduce(
                    out=sums[:],
                    in_=psum_t[:].rearrange("p (s c) -> p s c", s=nslot),
                    axis=Ax.X, op=Alu.add,
                )
            counts = small.tile([_P, nslot], f32)
            nc.vector.tensor_reduce(out=counts[:], in_=ind3, axis=Ax.X, op=Alu.add)
            nc.vector.tensor_scalar_max(counts[:], counts[:], 1.0)
            w_t = small.tile([_P, nslot], f32)
            nc.vector.reciprocal(w_t[:], counts[:])
            means = small.tile([_P, nslot], f32)
            nc.vector.tensor_tensor(
                out=means[:], in0=sums[:], in1=w_t[:], op=Alu.mult
            )

            x_t = big.tile([_P, FREE], f32)
            x3 = x_t[:].rearrange("p (s l) -> p s l", s=nslot)
            nc.vector.tensor_tensor(
                out=x3, in0=v3, in1=means[:].to_broadcast([_P, nslot, L]),
                op=Alu.subtract,
            )
            devs = small.tile([_P, nslot], f32)
            nc.vector.tensor_reduce(
                out=devs[:], in_=x3, axis=Ax.X, op=Alu.add,
                apply_absolute_value=True,
            )
            # pads were gathered as 0, so each contributed |0 - mean| to devs;
            # subtract the known pad contribution (L - count) * |mean|.
            npad = small.tile([_P, nslot], f32)
            nc.vector.tensor_scalar(
                out=npad[:], in0=counts[:], scalar1=-1.0, scalar2=float(L),
                op0=Alu.mult, op1=Alu.add,
            )
            absm = small.tile([_P, nslot], f32)
            nc.vector.tensor_scalar(
                out=absm[:], in0=means[:], scalar1=-1.0, scalar2=None, op0=Alu.mult
            )
            nc.vector.tensor_tensor(
                out=absm[:], in0=absm[:], in1=means[:], op=Alu.max
            )
            nc.vector.tensor_tensor(
                out=npad[:], in0=npad[:], in1=absm[:], op=Alu.mult
            )
            nc.vector.tensor_tensor(
                out=devs[:], in0=devs[:], in1=npad[:], op=Alu.subtract
            )
            contrib = small.tile([_P, nslot], f32)
            nc.vector.tensor_tensor(
                out=contrib[:], in0=devs[:], in1=w_t[:], op=Alu.mult
            )
            part = small.tile([_P, 1], f32)
            nc.vector.tensor_reduce(
                out=part[:], in_=contrib[:], axis=Ax.X, op=Alu.add
            )
            nc.sync.dma_start(out=out_d[:, :], in_=part[:])
    return nc


_CACHE = {}


def _get_nc(key):
    if key not in _CACHE:
        _CACHE[key] = _build_nc(*key)
    return _CACHE[key]


def _pack(input, rows, cols, seg_ids, num_paths):
    """Host-side sharding: one image per core, segments packed into a
    [ncore, 128, nslot*L] padded slot grid."""
    B, C, H, W = input.shape
    ppi = num_paths // B  # paths (segments) per image
    npix = rows.shape[0]

    bnd = np.searchsorted(seg_ids, np.arange(num_paths + 1)).astype(np.int64)
    seg_lens = np.diff(bnd)
    nslot = int(np.ceil(ppi / _P))
    lmax = int(seg_lens.max()) if npix else 1
    L = max(128, int(np.ceil(lmax / 128.0)) * 128)
    FREE = nslot * L

    s = np.arange(num_paths)
    core = s // ppi
    local = s % ppi
    part = local % _P
    slot = local // _P
    base = ((core * _P + part) * np.int64(nslot) + slot) * L
    dest = np.repeat(base, seg_lens) + (
        np.arange(npix, dtype=np.int64) - np.repeat(bnd[:-1], seg_lens)
    )
    ind_p = np.zeros(B * _P * FREE, np.uint8)
    ind_p[dest] = 1
    # Pixel values in slot layout.  This lookup runs on the host: the
    # toolchain's walrus build mis-lowers sub-row indirect DMA (one
    # descriptor per contiguous dest run, only the run-start offset is
    # honored), so a per-element device gather is not expressible; all
    # reductions stay on device.
    core_of = np.repeat(core, seg_lens)
    v_p = np.zeros(B * _P * FREE, np.float32)
    v_p[dest] = input[core_of, 0, rows, cols]
    return (v_p.reshape(B, _P, FREE), ind_p.reshape(B, _P, FREE),
            nslot, L, H * W + 128)


def kernel(input, rows, cols, seg_ids, _trace=False, _num_paths=_NUM_PATHS):
    from concourse.bass_utils import run_bass_kernel_spmd

    input = np.ascontiguousarray(np.asarray(input, np.float32))
    rows = np.ascontiguousarray(np.asarray(rows, np.int32))
    cols = np.ascontiguousarray(np.asarray(cols, np.int32))
    seg_ids = np.ascontiguousarray(np.asarray(seg_ids, np.int32))
    B, C, H, W = input.shape

    v_p, ind_p, nslot, L, ntot = _pack(input, rows, cols, seg_ids, _num_paths)
    chunk = L // 2 if (L % 2 == 0 and L >= 512) else L
    nc = _get_nc((nslot, L, ntot, W, H, chunk))
    in_maps = [
        {"vP": v_p[i], "indP": ind_p[i]} for i in range(B)
    ]
    res = run_bass_kernel_spmd(nc, in_maps, core_ids=list(range(B)), trace=_trace)
    total = sum(float(r["out"].sum()) for r in res.results)
    out = np.float32(total / B)
    if _trace:
        return out, res
    return out



# revision 3
# speedup vs baseline: 1.6884x; 1.6884x over previous
"""CIGLoss (segment_reduce) Trainium2 kernel.

Strategy (data-parallel over batch, per the sharding hint):
  - Each of the 8 NeuronCores owns one image and that image's pixel list
    (segments are image-local: seg // 500 == image).
  - Host-side sharding packs each image's ~500 segments into a
    [128 partitions, NSLOT slots, L] padded grid (one whole segment per
    slot), values cast to fp16 (loss tolerance 2e-2 >> fp16 error).
    Pad entries are 0.  The value lookup input[b,0,row,col] happens
    during host packing (this toolchain's walrus mis-lowers per-element
    indirect DMA — verified by hardware probes in a previous session).
  - Per-segment counts are metadata (a function of seg_ids only); the
    host ships w=1/max(count,1), -w, and npad=L-count as a tiny f32
    side tensor instead of an on-device indicator reduction.
  - On device, per slot s (fused DVE ops, fp16 = 2x/4x perf modes):
        sums_s  = accum_add(v_s * 1.0)              (tensor_scalar+accum)
        negmean = (sums_h0 + sums_h1) * (-w)        (scalar_tensor_tensor)
        d       = v_s + negmean                     (tensor_scalar, 4x)
        devs_s  = accum_add(abs_max(d, 0))          (tensor_scalar+accum)
    pads contributed |0 - mean| each, so subtract npad*|mean| and scale:
        contrib = (devs - npad*|mean|) * w
  - Host sums the 8 cores' [128, nslot] partials and divides by B.
"""

import numpy as np

_NUM_PATHS = 4000
_P = 128  # SBUF partitions


def _build_nc(nslot: int, L: int, cps: int, dev_eng: tuple, sums_eng: tuple):
    import concourse.bacc as bacc
    import concourse.bass as bass
    import concourse.tile as tile
    from concourse import mybir

    f32 = mybir.dt.float32
    f16 = mybir.dt.float16
    Alu = mybir.AluOpType
    Act = mybir.ActivationFunctionType
    FREE = nslot * L
    C = L // cps  # DMA chunk length (per slot piece)
    assert L % cps == 0

    nc = bacc.Bacc("TRN2", debug=False)
    v_d = nc.dram_tensor("vP", [_P, FREE], f16, kind="ExternalInput")
    sm_d = nc.dram_tensor("smP", [_P, 3 * nslot], f32, kind="ExternalInput")
    out_d = nc.dram_tensor("out", [_P, nslot], f32, kind="ExternalOutput")

    with tile.TileContext(nc) as tc:
        with (
            tc.tile_pool(name="big", bufs=1) as big,
            tc.tile_pool(name="small", bufs=1) as small,
        ):
            sm_t = small.tile([_P, 3 * nslot], f32)
            nc.sync.dma_start(out=sm_t[:], in_=sm_d[:, :])
            negw = sm_t[:, 0:nslot]
            w = sm_t[:, nslot:2 * nslot]
            npad = sm_t[:, 2 * nslot:3 * nslot]

            v_t = big.tile([_P, FREE], f16)
            d_t = big.tile([_P, L], f16)
            e_t = big.tile([_P, L], f16)
            sums2 = small.tile([_P, nslot * cps], f32)
            negmean = small.tile([_P, nslot], f32)
            devs = small.tile([_P, nslot], f32)
            tpad = small.tile([_P, nslot], f32)
            contrib = small.tile([_P, nslot], f32)

            for s in range(nslot):
                for j in range(cps):
                    a = s * L + j * C
                    b = a + C
                    nc.sync.dma_start(out=v_t[:, a:b], in_=v_d[:, a:b])
                    k = s * cps + j
                    if sums_eng[s] == "dve":
                        nc.vector.tensor_scalar(
                            out=d_t[:, j * C:(j + 1) * C], in0=v_t[:, a:b],
                            scalar1=1.0, scalar2=None, op0=Alu.mult,
                            op1=Alu.add, accum_out=sums2[:, k:k + 1],
                        )
                    else:  # act
                        nc.scalar.activation(
                            out=d_t[:, j * C:(j + 1) * C], in_=v_t[:, a:b],
                            func=Act.Copy, accum_out=sums2[:, k:k + 1],
                        )
                # negmean_s = -(sum of chunk partials) * w
                if cps == 1:
                    nc.vector.scalar_tensor_tensor(
                        out=negmean[:, s:s + 1], in0=sums2[:, s:s + 1],
                        scalar=-1.0, in1=w[:, s:s + 1],
                        op0=Alu.mult, op1=Alu.mult,
                    )
                elif cps == 2:
                    nc.vector.scalar_tensor_tensor(
                        out=negmean[:, s:s + 1], in0=sums2[:, 2 * s:2 * s + 1],
                        scalar=sums2[:, 2 * s + 1:2 * s + 2],
                        in1=negw[:, s:s + 1], op0=Alu.add, op1=Alu.mult,
                    )
                else:
                    raise ValueError(cps)
                # Sum_real |v-m| == 2*Sum_real relu(v-m) (real devs sum to ~0),
                # so accumulate R_s = Sum_slot relu(v + negmean); pads (v=0)
                # contribute relu(-m) each, corrected in the tail.
                sl = v_t[:, s * L:(s + 1) * L]
                if dev_eng[s] == "dve":
                    nc.vector.tensor_scalar(
                        out=d_t[:], in0=sl, scalar1=negmean[:, s:s + 1],
                        scalar2=0.0, op0=Alu.add, op1=Alu.max,
                    )
                    nc.vector.tensor_scalar(
                        out=e_t[:], in0=d_t[:], scalar1=1.0, scalar2=None,
                        op0=Alu.mult, op1=Alu.add,
                        accum_out=devs[:, s:s + 1],
                    )
                else:  # act
                    nc.scalar.activation(
                        out=d_t[:], in_=sl, func=Act.Relu,
                        bias=negmean[:, s:s + 1], scale=1.0,
                        accum_out=devs[:, s:s + 1],
                    )

            # tail: contrib = 2*(R - npad*relu(-mean)) * w  (tiny [P, nslot])
            nc.vector.scalar_tensor_tensor(
                out=tpad[:], in0=negmean[:], scalar=0.0, in1=npad,
                op0=Alu.max, op1=Alu.mult,
            )
            nc.vector.tensor_tensor(
                out=tpad[:], in0=devs[:], in1=tpad[:], op=Alu.subtract,
            )
            nc.vector.scalar_tensor_tensor(
                out=contrib[:], in0=tpad[:], scalar=2.0, in1=w,
                op0=Alu.mult, op1=Alu.mult,
            )
            nc.sync.dma_start(out=out_d[:, :], in_=contrib[:])

    nc.finalize()
    return nc


_CACHE = {}


def _get_nc(key):
    if key not in _CACHE:
        _CACHE[key] = _build_nc(*key)
    return _CACHE[key]


def _pack(input, rows, cols, seg_ids, num_paths):
    """Host-side sharding: one image per core, segments packed into a
    [ncore, 128, nslot*L] padded slot grid (fp16), plus per-slot
    metadata [-w | w | npad] derived from seg_ids alone."""
    B, C, H, W = input.shape
    ppi = num_paths // B  # paths (segments) per image
    npix = rows.shape[0]

    bnd = np.searchsorted(seg_ids, np.arange(num_paths + 1)).astype(np.int64)
    seg_lens = np.diff(bnd)
    nslot = int(np.ceil(ppi / _P))
    lmax = int(seg_lens.max()) if npix else 1
    L = max(8, int(np.ceil(lmax / 8.0)) * 8)
    FREE = nslot * L

    s = np.arange(num_paths)
    core = s // ppi
    local = s % ppi
    part = local % _P
    slot = local // _P
    base = ((core * _P + part) * np.int64(nslot) + slot) * L
    dest = np.repeat(base, seg_lens) + (
        np.arange(npix, dtype=np.int64) - np.repeat(bnd[:-1], seg_lens)
    )
    core_of = np.repeat(core, seg_lens)
    v_p = np.zeros(B * _P * FREE, np.float16)
    v_p[dest] = input[core_of, 0, rows, cols]

    counts = np.zeros((B, _P, nslot), np.float32)
    counts[core, part, slot] = seg_lens
    w = 1.0 / np.maximum(counts, 1.0)
    sm = np.concatenate([-w, w, np.float32(L) - counts], axis=-1)
    return v_p.reshape(B, _P, FREE), np.ascontiguousarray(sm), nslot, L


def kernel(input, rows, cols, seg_ids, _trace=False, _num_paths=_NUM_PATHS,
           _cps=2, _dev_eng=None, _sums_eng=None):
    from concourse.bass_utils import run_bass_kernel_spmd

    input = np.ascontiguousarray(np.asarray(input, np.float32))
    rows = np.ascontiguousarray(np.asarray(rows, np.int32))
    cols = np.ascontiguousarray(np.asarray(cols, np.int32))
    seg_ids = np.ascontiguousarray(np.asarray(seg_ids, np.int32))
    B, C, H, W = input.shape

    v_p, sm, nslot, L = _pack(input, rows, cols, seg_ids, _num_paths)
    dev_eng = tuple(_dev_eng) if _dev_eng else ("dve",) * nslot
    sums_eng = tuple(_sums_eng) if _sums_eng else ("dve",) * nslot
    nc = _get_nc((nslot, L, _cps, dev_eng, sums_eng))
    in_maps = [{"vP": v_p[i], "smP": sm[i]} for i in range(B)]
    res = run_bass_kernel_spmd(nc, in_maps, core_ids=list(range(B)), trace=_trace)
    total = sum(float(r["out"].sum()) for r in res.results)
    out = np.float32(total / B)
    if _trace:
        return out, res
    return out


# revision 4
# speedup vs baseline: 2.1909x; 1.2976x over previous
"""CIGLoss (segment_reduce) Trainium2 kernel.

Strategy (data-parallel over batch, per the sharding hint):
  - Each of the 8 NeuronCores owns one image and that image's pixel list
    (segments are image-local: seg // 500 == image).
  - Host-side sharding packs each image's ~500 segments into a
    [128 partitions, NSLOT slots, L] padded grid (one whole segment per
    slot), values cast to fp16/fp8 (loss tolerance 2e-2 >> cast error).
    Pad entries are 0.  The value lookup input[b,0,row,col] happens
    during host packing (this toolchain's walrus mis-lowers per-element
    indirect DMA — verified by hardware probes in a previous session).
  - Per-segment counts are metadata (a function of seg_ids only); the
    host ships w=1/max(count,1), -w, npad=L-count as a tiny f32 tensor.
  - On device, Sum_real |v-m| == 2*Sum_real relu(v-m) (real deviations
    sum to ~0), and pads (v=0) contribute relu(-m) each:
        sums_s  = accum_add(Copy(v_s))              on ACT (idle engine)
        negmean = -sums*w                           tiny DVE op
        R_s     = accum_add((v_s + negmean) max 0)  fused DVE STT
        contrib = 2*(R - npad*relu(negmean)) * w
  - Host sums the 8 cores' [128, nslot] partials and divides by B.
"""

import numpy as np

_NUM_PATHS = 4000
_P = 128  # SBUF partitions


def _build_nc(nslot: int, L: int, vdt: str, sums_eng: tuple, dev_mode: str,
              split_dma: bool):
    import concourse.bacc as bacc
    import concourse.bass as bass
    import concourse.tile as tile
    from concourse import mybir

    f32 = mybir.dt.float32
    fv = {"f16": mybir.dt.float16, "f8": mybir.dt.float8e4}[vdt]
    f16 = mybir.dt.float16
    Alu = mybir.AluOpType
    Act = mybir.ActivationFunctionType
    FREE = nslot * L

    nc = bacc.Bacc("TRN2", debug=False)
    v_d = nc.dram_tensor("vP", [_P, FREE], fv, kind="ExternalInput")
    sm_d = nc.dram_tensor("smP", [_P, 3 * nslot], f32, kind="ExternalInput")
    out_d = nc.dram_tensor("out", [_P, nslot], f32, kind="ExternalOutput")

    with tile.TileContext(nc) as tc:
        with (
            tc.tile_pool(name="big", bufs=1) as big,
            tc.tile_pool(name="small", bufs=1) as small,
        ):
            sm_t = small.tile([_P, 3 * nslot], f32)
            (nc.scalar if split_dma else nc.sync).dma_start(
                out=sm_t[:], in_=sm_d[:, :])
            negw = sm_t[:, 0:nslot]
            w = sm_t[:, nslot:2 * nslot]
            npad = sm_t[:, 2 * nslot:3 * nslot]

            v_t = big.tile([_P, FREE], fv)
            a_t = big.tile([_P, L], f16)   # ACT sums scratch
            d_t = big.tile([_P, L], f16)   # DVE dev scratch
            z_t = big.tile([_P, L], f16)   # zeros for the STT max
            if dev_mode == "stt":
                nc.gpsimd.memset(z_t[:], 0.0)
            sums = small.tile([_P, nslot], f32)
            negmean = small.tile([_P, nslot], f32)
            devs = small.tile([_P, nslot], f32)
            tpad = small.tile([_P, nslot], f32)
            contrib = small.tile([_P, nslot], f32)

            for s in range(nslot):
                a, b = s * L, (s + 1) * L
                eng = nc.scalar if (split_dma and s % 2) else nc.sync
                eng.dma_start(out=v_t[:, a:b], in_=v_d[:, a:b])
                sl = v_t[:, a:b]
                if sums_eng[s] == "act":
                    nc.scalar.activation(
                        out=a_t[:], in_=sl, func=Act.Copy,
                        accum_out=sums[:, s:s + 1],
                    )
                else:  # dve
                    nc.vector.tensor_scalar(
                        out=a_t[:], in0=sl, scalar1=1.0, scalar2=None,
                        op0=Alu.mult, op1=Alu.add,
                        accum_out=sums[:, s:s + 1],
                    )
                nc.vector.scalar_tensor_tensor(
                    out=negmean[:, s:s + 1], in0=sums[:, s:s + 1],
                    scalar=-1.0, in1=w[:, s:s + 1],
                    op0=Alu.mult, op1=Alu.mult,
                )
                if dev_mode == "stt":
                    nc.vector.scalar_tensor_tensor(
                        out=d_t[:], in0=sl, scalar=negmean[:, s:s + 1],
                        in1=z_t[:], op0=Alu.add, op1=Alu.max,
                        accum_out=devs[:, s:s + 1],
                    )
                else:  # 2op
                    nc.vector.tensor_scalar(
                        out=d_t[:], in0=sl, scalar1=negmean[:, s:s + 1],
                        scalar2=0.0, op0=Alu.add, op1=Alu.max,
                    )
                    nc.vector.tensor_scalar(
                        out=d_t[:], in0=d_t[:], scalar1=1.0, scalar2=None,
                        op0=Alu.mult, op1=Alu.add,
                        accum_out=devs[:, s:s + 1],
                    )

            # tail: contrib = 2*(R - npad*relu(negmean)) * w  (tiny [P,nslot])
            nc.vector.scalar_tensor_tensor(
                out=tpad[:], in0=negmean[:], scalar=0.0, in1=npad,
                op0=Alu.max, op1=Alu.mult,
            )
            nc.vector.tensor_tensor(
                out=tpad[:], in0=devs[:], in1=tpad[:], op=Alu.subtract,
            )
            nc.vector.scalar_tensor_tensor(
                out=contrib[:], in0=tpad[:], scalar=2.0, in1=w,
                op0=Alu.mult, op1=Alu.mult,
            )
            nc.sync.dma_start(out=out_d[:, :], in_=contrib[:])

    nc.finalize()
    return nc


_CACHE = {}


def _get_nc(key):
    if key not in _CACHE:
        _CACHE[key] = _build_nc(*key)
    return _CACHE[key]


def _pack(input, rows, cols, seg_ids, num_paths, vdt):
    """Host-side sharding: one image per core, segments packed into a
    [ncore, 128, nslot*L] padded slot grid, plus per-slot metadata
    [-w | w | npad] derived from seg_ids alone."""
    from concourse import mybir

    B, C, H, W = input.shape
    ppi = num_paths // B  # paths (segments) per image
    npix = rows.shape[0]

    bnd = np.searchsorted(seg_ids, np.arange(num_paths + 1)).astype(np.int64)
    seg_lens = np.diff(bnd)
    nslot = int(np.ceil(ppi / _P))
    lmax = int(seg_lens.max()) if npix else 1
    L = max(8, int(np.ceil(lmax / 8.0)) * 8)
    FREE = nslot * L

    s = np.arange(num_paths)
    core = s // ppi
    local = s % ppi
    part = local % _P
    slot = local // _P
    base = ((core * _P + part) * np.int64(nslot) + slot) * L
    dest = np.repeat(base, seg_lens) + (
        np.arange(npix, dtype=np.int64) - np.repeat(bnd[:-1], seg_lens)
    )
    core_of = np.repeat(core, seg_lens)
    np_dt = mybir.dt.np({"f16": mybir.dt.float16,
                         "f8": mybir.dt.float8e4}[vdt])
    v_p = np.zeros(B * _P * FREE, np_dt)
    v_p[dest] = input[core_of, 0, rows, cols]

    counts = np.zeros((B, _P, nslot), np.float32)
    counts[core, part, slot] = seg_lens
    w = 1.0 / np.maximum(counts, 1.0)
    sm = np.concatenate([-w, w, np.float32(L) - counts], axis=-1)
    return v_p.reshape(B, _P, FREE), np.ascontiguousarray(sm), nslot, L


def kernel(input, rows, cols, seg_ids, _trace=False, _num_paths=_NUM_PATHS,
           _vdt="f16", _sums_eng=None, _dev_mode="stt", _split_dma=True):
    from concourse.bass_utils import run_bass_kernel_spmd

    input = np.ascontiguousarray(np.asarray(input, np.float32))
    rows = np.ascontiguousarray(np.asarray(rows, np.int32))
    cols = np.ascontiguousarray(np.asarray(cols, np.int32))
    seg_ids = np.ascontiguousarray(np.asarray(seg_ids, np.int32))
    B, C, H, W = input.shape

    v_p, sm, nslot, L = _pack(input, rows, cols, seg_ids, _num_paths, _vdt)
    sums_eng = tuple(_sums_eng) if _sums_eng else ("act",) * nslot
    nc = _get_nc((nslot, L, _vdt, sums_eng, _dev_mode, _split_dma))
    in_maps = [{"vP": v_p[i], "smP": sm[i]} for i in range(B)]
    res = run_bass_kernel_spmd(nc, in_maps, core_ids=list(range(B)), trace=_trace)
    total = sum(float(r["out"].sum()) for r in res.results)
    out = np.float32(total / B)
    if _trace:
        return out, res
    return out
